# revision 3
# baseline (speedup 1.0000x reference)
"""AdaptiveTokenMixer Trainium2 kernel v4 (8 NeuronCores, pure data parallel).

Front half (alpha) in [120, (block b, tap p)] layout: host folds the
valid-mask into the packed dt windows (invalid -> +/-LARGE so exp -> 0
exactly), premultiplies softmax(w)*b/(1-b) into the vf windows, and
prepacks cbs1 = 1 + sum_p vfbw; device work is one sub + exp + reduce +
five small ops, af = (e + vfbw*s) * r with r = 1/max(s*cbs1, 1e-8).

Back half keeps the banded-matmul pipeline: one skewed DMA per chunk
scatters af rows (16B runs) into the DRAM scratch forming
W^T[m, k] = alpha[120b+m, k-m]; one DMA-transpose XBAR per chunk (Sync
queue) loads W[k, m] blocks; one 128x120 @ 128x256 bf16 matmul per block;
paired PSUM evictions (DVE/ACT alternating).

I/O uses partition-major DRAM layouts (host im2col / un-im2col): x loads
are 128 descriptors of ~9KB, stores 120 per chunk, and comb leads the
sync ring so alpha starts as early as possible.

Self-contained: hardcodes shapes for B=8, N=4096, d=256, K=8.
"""
import numpy as np
import ml_dtypes

import concourse.bass as bass
import concourse.bacc as bacc
import concourse.mybir as mybir
from concourse import tile
from concourse.bass_utils import run_bass_kernel_spmd

B, N, D, K = 8, 4096, 256, 8
BLK = 120                      # output positions per block
NB = (N + BLK - 1) // BLK      # 35 blocks
NPAD = 4224
KW = 128                       # k-window per block
WBLK = KW * KW                 # W scratch elements per block
AFR = NB * K                   # af row width = 280
ODTB = AFR                     # comb col offsets: dts | dtb | vfbw | cbs1
OVFB = AFR + NB
OCBS = 2 * AFR + NB
CW = 2 * AFR + 2 * NB          # 630
LARGE = 1.0e6
XCHUNKS = [(0, 18), (18, 17)]
CHUNKS = [(0, 5), (5, 10), (15, 10), (25, 10)]
SCHUNKS = [(0, 12), (12, 12), (24, 11)]

_CACHE = {}


def _build():
    nc = bacc.Bacc("TRN2", target_bir_lowering=False, debug=False,
                   num_devices=B)
    f32 = mybir.dt.float32
    bf16 = mybir.dt.bfloat16

    x_t = nc.dram_tensor("x", [128, NB * D], bf16, kind="ExternalInput")
    comb_t = nc.dram_tensor("comb", [BLK, CW], f32, kind="ExternalInput")
    wz_t = nc.dram_tensor("wz", [NB * WBLK], bf16, kind="ExternalInput")
    out_t = nc.dram_tensor("out", [BLK, NB * D], bf16, kind="ExternalOutput")

    with tile.TileContext(nc) as tc:
        with tc.tile_pool(name="alph", bufs=1) as apool, \
             tc.tile_pool(name="big", bufs=1) as bpool, \
             tc.tile_pool(name="ps", bufs=3, space="PSUM") as pspool:

            # ---- input loads (comb leads the sync ring) ----
            comb = apool.tile([BLK, CW], f32)
            nc.sync.dma_start(comb[:], bass.AP(comb_t, 0, [[CW, BLK], [1, CW]]))
            x_all = bpool.tile([128, NB, D], bf16)
            for qe, (j0, nj) in zip((nc.sync, nc.gpsimd), XCHUNKS):
                qe.dma_start(
                    x_all[:, j0:j0 + nj, :],
                    bass.AP(x_t, j0 * D, [[NB * D, 128], [1, nj * D]]))

            # ---- alpha stage ----
            def cols(c0, dims):
                return bass.AP(comb.tensor, comb.offset + c0,
                               [comb[:].ap[0]] + dims)

            td = apool.tile([BLK, AFR], f32)
            nc.vector.tensor_tensor(
                td[:], cols(0, [[K, NB], [1, K]]),
                cols(ODTB, [[1, NB], [0, K]]), mybir.AluOpType.subtract)
            e = apool.tile([BLK, NB, K], bf16)
            nc.scalar.activation(e[:], td[:],
                                 mybir.ActivationFunctionType.Exp, scale=-1.0)
            s = apool.tile([BLK, NB], f32)
            nc.vector.tensor_reduce(s[:], e[:], mybir.AxisListType.X,
                                    mybir.AluOpType.add)
            sa = apool.tile([BLK, NB], f32)
            nc.vector.tensor_tensor(sa[:], s[:], cols(OCBS, [[1, NB]]),
                                    mybir.AluOpType.mult)
            nc.vector.tensor_scalar(sa[:], sa[:], 1e-8, None,
                                    mybir.AluOpType.max)
            r = apool.tile([BLK, NB], f32)
            nc.vector.reciprocal(r[:], sa[:])
            u = apool.tile([BLK, NB, K], bf16)
            nc.vector.tensor_tensor(
                u[:], cols(OVFB, [[K, NB], [1, K]]),
                bass.AP(s.tensor, s.offset, [s[:].ap[0], [1, NB], [0, K]]),
                mybir.AluOpType.mult)
            v = apool.tile([BLK, NB, K], bf16)
            nc.vector.tensor_tensor(v[:], u[:], e[:], mybir.AluOpType.add)
            af = apool.tile([BLK, NB, K], bf16)
            nc.vector.tensor_tensor(
                af[:], v[:],
                bass.AP(r.tensor, r.offset, [r[:].ap[0], [1, NB], [0, K]]),
                mybir.AluOpType.mult)

            # ---- pipeline per chunk: skew -> xbar -> matmuls ----
            out_all = bpool.tile([128, NB, D], bf16)
            w_all = bpool.tile([128, NB, KW], bf16)
            evict = [nc.vector.tensor_copy, nc.scalar.copy]
            pts = {}

            def skew(j0, nj, qe):  # af[m, b, p] -> wz[b][m, m+p]
                qe.dma_start(
                    bass.AP(wz_t, j0 * WBLK,
                            [[KW + 1, BLK], [WBLK, nj], [1, K]]),
                    bass.AP(af.tensor, af.offset + j0 * K,
                            [[AFR, BLK], [K, nj], [1, K]]))

            def xbar(ci, qe):
                j0, nj = CHUNKS[ci]
                qe.dma_start(
                    w_all[:, j0:j0 + nj, :],
                    bass.AP(wz_t, j0 * WBLK, [[KW, nj * KW], [1, KW]]),
                    transpose=True)

            SKQ = [nc.sync, nc.scalar, nc.gpsimd, nc.sync]

            def run_chunk(ci):
                j0, nj = CHUNKS[ci]
                skew(j0, nj, SKQ[ci])
                xbar(ci, nc.sync)
                for jj in range(nj):
                    b = j0 + jj
                    pi = b // 4
                    if b % 4 == 0:
                        pt = pspool.tile([BLK, 4, D], f32, tag="mm",
                                         name=f"pt{pi}")
                        pts[pi] = pt
                    pt = pts[pi]
                    nc.tensor.matmul(pt[:, b % 4, :], w_all[:, b, :BLK],
                                     x_all[:, b, :])
                    if b % 4 == 3:
                        evict[pi % 2](out_all[:BLK, b - 3:b + 1, :], pt[:])
                    elif b == NB - 1:
                        evict[pi % 2](out_all[:BLK, b - 2:b + 1, :],
                                      pt[:, :3, :])

            for ci in range(len(CHUNKS)):
                run_chunk(ci)
            for j0, nj in SCHUNKS:
                nc.sync.dma_start(
                    bass.AP(out_t, j0 * D, [[NB * D, BLK], [1, nj * D]]),
                    out_all[:BLK, j0:j0 + nj, :])
    nc.compile()
    return nc


def _get_nc():
    if "nc" not in _CACHE:
        _CACHE["nc"] = _build()
    return _CACHE["nc"]


def _make_in_maps(x, delta_times, valid_mask, w, beta):
    w64 = w.astype(np.float64)
    wsm = np.exp(w64 - w64.max())
    wsm /= wsm.sum()
    b = 1.0 / (1.0 + np.exp(-float(beta[0])))
    bw = (b / (1.0 - b) * wsm).astype(np.float32)
    wz = np.zeros(NB * WBLK, np.float32).astype(ml_dtypes.bfloat16)

    m = np.arange(BLK)
    n0 = 120 * np.arange(NB)[None, :] + m[:, None]
    nd = n0[:, :, None] + np.arange(K)[None, None, :]
    xidx = np.arange(128)[:, None] + BLK * np.arange(NB)[None, :]

    in_maps = []
    for i in range(B):
        dtp = np.zeros(NPAD, np.float32)
        dtp[:N] = delta_times[i]
        vfp = np.zeros(NPAD, np.float32)
        vfp[:N] = valid_mask[i].astype(np.float32)
        comb = np.empty((BLK, CW), np.float32)
        comb[:, :AFR] = (dtp[nd] + LARGE * (1.0 - vfp[nd])).reshape(BLK, AFR)
        comb[:, ODTB:ODTB + NB] = dtp[n0] - LARGE * (1.0 - vfp[n0])
        vfbw = vfp[nd] * bw[None, None, :]
        comb[:, OVFB:OVFB + AFR] = vfbw.reshape(BLK, AFR)
        comb[:, OCBS:OCBS + NB] = 1.0 + vfbw.sum(-1)
        xp = np.zeros((NPAD, D), np.float32)
        xp[:N] = x[i]
        in_maps.append({
            "x": np.ascontiguousarray(
                xp[xidx].reshape(128, NB * D)).astype(ml_dtypes.bfloat16),
            "comb": comb,
            "wz": wz,
        })
    return in_maps


def _execute(in_maps, trace=False, **kw):
    nc = _get_nc()
    return run_bass_kernel_spmd(nc, in_maps, core_ids=list(range(B)),
                                trace=trace, **kw)


def kernel(x, delta_times, valid_mask, w, beta):
    in_maps = _make_in_maps(x, delta_times, valid_mask, w, beta)
    kr = _execute(in_maps, trace=False)
    outs = []
    for i in range(B):
        o = kr.results[i]["out"].reshape(BLK, NB, D).astype(np.float32)
        outs.append(o.transpose(1, 0, 2).reshape(NB * BLK, D)[:N])
    return np.stack(outs, axis=0)
